# revision 3
# baseline (speedup 1.0000x reference)
"""TRN2 Bass kernel for CP-decoding (nn_CPDecoding), v4: SWDGE NN-row gather.

Replaces v3's Pool-bound ap_gather (27ns/idx) with SWDGE dma_gather of 48B
bf16 NN-table rows from HBM: Pool only generates descriptors (~1.3us per
768-idx chunk), transfers run on the DMA engines. Table [32640, 12] int32 =
24 bf16 components per K=85 oversampled grid row (3 dims concatenated).

Chunk = 768 idxs = 256 points x 3 dims. Gather-list position
i = (d*2+h)*128 + p' maps to point p = 2*p'+h of the chunk and dim d, so
each point's 3 rows land on SBUF partition p' at slots d*2+h (SWDGE writes
slot i to partition i%128, free-slot i//128). Products and the 24-component
sum are plain elementwise DVE + free-axis reduce; no matmul, no transposes.
Host pre-wraps coords into the replicated 16-partition idx layout.
"""

import sys

try:
    import concourse.bass  # noqa: F401
except Exception:
    sys.path.insert(0, "/opt/trn_rl_repo")

import numpy as np

import concourse.bacc as bacc
import concourse.bass as bass
import concourse.mybir as mybir
import concourse.tile as tile

F32 = mybir.dt.float32
I16 = mybir.dt.int16
I32 = mybir.dt.int32
BF16 = mybir.dt.bfloat16
COPY = mybir.ActivationFunctionType.Copy
ALU = mybir.AluOpType

N_TOTAL = 2097152
N_CORES = 8
N_PER_CORE = N_TOTAL // N_CORES   # 262144
R = 256
C = 24

K85 = 85
JMAX = 10838
PAD = 10880
NE = 3 * PAD              # 32640 table rows
SCALE = 127.5 * K85

CP = 256                  # points per gather chunk
CI = 3 * CP               # idxs per chunk (768)
CW = CI // 16             # wrapped idx cols per chunk (48)
KC = 16                   # chunks per tile
TP = KC * CP              # points per tile (4096)


def _bf16_bits(x: np.ndarray) -> np.ndarray:
    b = np.ascontiguousarray(x, np.float32).view(np.uint32)
    return (b + 0x7FFF + ((b >> 16) & 1)) >> 16


def build_htab(line_coef: np.ndarray) -> np.ndarray:
    """[3,24,256] f32 -> [NE, 64] int32 (256B rows): 24 bf16 + pad."""
    lc = np.ascontiguousarray(line_coef, dtype=np.float32)
    assert lc.shape == (3, C, R)
    j = np.arange(JMAX + 1)
    pos = 127.5 + j / K85
    i0 = np.minimum(np.floor(pos), R - 1).astype(np.int64)
    i1 = np.minimum(i0 + 1, R - 1)
    w = (pos - np.floor(pos)).astype(np.float32)
    tab = np.zeros((3, PAD, 64), np.uint32)
    for dp in range(3):  # dp = coords column; line index is 2-dp
        L = lc[2 - dp]                                   # [24, 256]
        v = L[:, i0] * (1.0 - w) + L[:, i1] * w          # [24, JMAX+1]
        lo = _bf16_bits(v[0::2]).T                       # [JMAX+1, 12]
        hi = _bf16_bits(v[1::2]).T
        tab[dp, : JMAX + 1, :12] = lo | (hi << 16)
    return tab.reshape(NE, 64).view(np.int32)


def wrap_coords(shard: np.ndarray, n_per_core: int = N_PER_CORE) -> np.ndarray:
    """[n,3] f32 -> [128, 3n/16]: chunk C position i=(d*2+h)*128+p' holds
    coord[C*256 + 2p'+h, d], wrapped (q=i%16, col=i//16), replicated x8."""
    chunks = n_per_core // CP
    c = np.ascontiguousarray(shard, np.float32).reshape(chunks, 128, 2, 3)
    a = c.transpose(0, 3, 2, 1).reshape(chunks, CI)      # [C, i] (d,h,p')
    w = a.reshape(chunks, CW, 16).transpose(2, 0, 1)     # [q, C, col]
    w = w.reshape(1, 16, chunks * CW)
    return np.ascontiguousarray(
        np.broadcast_to(w, (8, 16, chunks * CW)).reshape(128, chunks * CW))


def build_kernel(n_per_core: int = N_PER_CORE, bufs: int = 2):
    assert n_per_core % TP == 0
    tiles = n_per_core // TP
    mwt = KC * CW             # idx cols per tile (768)

    nc = bacc.Bacc("TRN2", target_bir_lowering=False, num_swdge_queues=4)
    coordsw = nc.dram_tensor("coordsw", [128, tiles * mwt], F32,
                             kind="ExternalInput")
    htab = nc.dram_tensor("htab", [NE, 64], I32, kind="ExternalInput")
    out = nc.dram_tensor("out", [n_per_core], F32, kind="ExternalOutput")

    with tile.TileContext(nc) as tc:
        with (
            tc.tile_pool(name="cdofs", bufs=1) as dpool,
            tc.tile_pool(name="sb", bufs=bufs) as pool,
            tc.tile_pool(name="gt", bufs=bufs) as gpool,
        ):
            dofs_t = dpool.tile([128, mwt], I16)
            dv = dofs_t[:, :].rearrange("p (k d s) -> p k d s", d=3, s=16)
            for d in range(3):
                nc.vector.memset(dv[:, :, d, :], d * PAD)

            tc.strict_bb_all_engine_barrier()
            ov = out.ap().rearrange("(k ph h) -> k ph h", ph=128, h=2)

            for t in range(tiles):
                cb = pool.tile([128, mwt], F32, tag="cb")
                nc.sync.dma_start(cb[:, :],
                                  coordsw.ap()[:, t * mwt:(t + 1) * mwt])

                v = pool.tile([128, mwt], F32, tag="v")
                nc.scalar.activation(v[:, :], cb[:, :], COPY,
                                     bias=0.5, scale=float(SCALE))
                r16 = pool.tile([128, mwt], I16, tag="r16")
                nc.vector.tensor_copy(r16[:, :], v[:, :])
                rf = pool.tile([128, mwt], F32, tag="rf")
                nc.vector.tensor_copy(rf[:, :], r16[:, :])
                g = pool.tile([128, mwt], F32, tag="g")
                nc.vector.tensor_tensor(out=g[:, :], in0=rf[:, :],
                                        in1=v[:, :], op=ALU.is_gt)
                nc.vector.tensor_tensor(out=rf[:, :], in0=rf[:, :],
                                        in1=g[:, :], op=ALU.subtract)
                idx = pool.tile([128, mwt], I16, tag="idx")
                nc.vector.tensor_copy(idx[:, :], rf[:, :])
                nc.vector.tensor_tensor(out=idx[:, :], in0=idx[:, :],
                                        in1=dofs_t[:, :], op=ALU.add)

                gt = gpool.tile([128, KC * 6, 64], I32, tag="gt")
                for k in range(KC):
                    nc.gpsimd.dma_gather(
                        gt[:, k * 6:(k + 1) * 6, :], htab.ap(),
                        idx[:, k * CW:(k + 1) * CW],
                        num_idxs=CI, num_idxs_reg=CI, elem_size=64,
                        queue_num=k % 4)

                # slot s=d*2+h: product over d, then 24-comp sum (free axis)
                gb = gt[:, :, :].bitcast(BF16) \
                    .rearrange("p (k d h) c -> p k d h c", d=3, h=2)[
                        :, :, :, :, 0:24]
                m1 = pool.tile([128, KC, 2, 24], F32, tag="m1")
                nc.vector.tensor_tensor(out=m1[:, :, :, :],
                                        in0=gb[:, :, 0, :, :],
                                        in1=gb[:, :, 1, :, :], op=ALU.mult)
                nc.vector.tensor_tensor(out=m1[:, :, :, :], in0=m1[:, :, :, :],
                                        in1=gb[:, :, 2, :, :], op=ALU.mult)
                res = pool.tile([128, KC, 2], F32, tag="res")
                nc.vector.tensor_reduce(out=res[:, :, :], in_=m1[:, :, :, :],
                                        axis=mybir.AxisListType.X, op=ALU.add)
                # point p = C*256 + 2p' + h -> dst (p' stride 8B)(C 1KB)(h 4B)
                nc.sync.dma_start(
                    ov[t * KC:(t + 1) * KC, :, :].rearrange("k ph h -> ph k h"),
                    res[:, :, :])
    nc.compile()
    return nc


_NC_CACHE = {}


def _get_nc():
    if N_PER_CORE not in _NC_CACHE:
        _NC_CACHE[N_PER_CORE] = build_kernel()
    return _NC_CACHE[N_PER_CORE]


def run(in_tensor: np.ndarray, line_coef: np.ndarray, trace: bool = False):
    from concourse.bass_utils import run_bass_kernel_spmd

    in_tensor = np.ascontiguousarray(in_tensor, dtype=np.float32)
    assert in_tensor.shape == (N_TOTAL, 3)
    htab = build_htab(np.asarray(line_coef))
    nc = _get_nc()
    shards = in_tensor.reshape(N_CORES, N_PER_CORE, 3)
    in_maps = [{"coordsw": wrap_coords(shards[i]), "htab": htab}
               for i in range(N_CORES)]
    res = run_bass_kernel_spmd(nc, in_maps, core_ids=list(range(N_CORES)),
                               trace=trace)
    out = np.concatenate([np.asarray(r["out"]) for r in res.results])
    return out, res


def kernel(in_tensor: np.ndarray, line_coef: np.ndarray) -> np.ndarray:
    out, _ = run(np.asarray(in_tensor), np.asarray(line_coef))
    return out
